# revision 4
# baseline (speedup 1.0000x reference)
"""Trainium2 Bass kernel for CAN multi-head message passing (GAT-style).

Strategy (vertex-cut by TARGET node, 8 cores):
  - Edges are sorted by target and sharded so core c owns target nodes
    [c*6250, (c+1)*6250). Each core fully computes its own output rows;
    no cross-core reduction is needed.
  - Phase A is SHARDED: core c computes x_msg = x @ W (4 heads) plus the
    per-node attention scalars s_n, t_n only for its own 6250 nodes
    (input = f16 x slice [128, 6250]), writes f16 rows
    [msg(256) | s(4) | t(4) | pad] (768B pitch) to a local DRAM chunk,
    then an on-device AllGather assembles the full 50000-row table on
    every core.  This keeps the host->device transfer per core at ~2.7MB
    instead of 34MB (the axon tunnel at ~40-50MB/s is the wall-clock
    bottleneck; device links are ~1000x faster).
  - Phase B: per 128-target-node window, per-edge rows are fetched with
    the GPSIMD dma_gather extended instruction.  The int16 index
    limitation is beaten by biasing indices by -32768 and pointing the
    gather base at row 32768: signed int16 offset arithmetic then
    addresses all 50k rows (verified on HW; the node table is stored in
    natural order).  Gathers are capped at 1024 indices per segment.
  - softmax (no max-subtraction needed: |z| <= ~10 for this data; a
    constant bias of -4 inside Exp guards fp16 range; constants cancel
    in softmax) and aggregation via one-hot matmuls: for each chunk of
    128 edges, a [128e x 128n] one-hot of local targets is built with a
    DVE is_equal and a single PE matmul accumulates both the weighted
    messages (256 cols) and the denominators (4 cols) into PSUM across
    all chunks of the window.
  - Gather index tiles are shipped compact ([16, nseg*64] int16) and
    replicated across the 8 gpsimd cores' partition groups on device;
    tgt-local ids ship as uint8; edge_vals are omitted when all-ones
    (the usual case); the output is written f16 and upcast on host.
"""
import sys
sys.path.insert(0, "/opt/trn_rl_repo")
import numpy as np

N_NODES = 50000
N_EDGES = 1600000
IN_CH = 128
OUT_CH = 64
N_HEADS = 4
HO = N_HEADS * OUT_CH          # 256
NCORES = 8
NPC = N_NODES // NCORES        # 6250 nodes per core
NW = 49                        # windows per core (48*128 + 106)
NTA = (NPC + 127) // 128       # phase-A tiles per core (49)
XROW = 384                     # fp16 elems per table row (768B): msg|s|t|pad
TROW = 128                     # fp16 elems per t-gather slice (256B)
SEG = 1024                     # max indices per dma_gather
SEGC = SEG // 128              # 8 chunks per segment
EXP_BIAS = -4.0
IDX_BIAS = 32768               # gather base at row 32768, idx = node - 32768


def _host_prep(x_source, edge_tgt, edge_src, edge_vals, weight, att_weight):
    perm = np.argsort(edge_tgt, kind="stable")
    tgt_s = np.asarray(edge_tgt)[perm].astype(np.int64)
    src_s = np.asarray(edge_src)[perm].astype(np.int64)
    val_s = np.asarray(edge_vals)[perm].astype(np.float32)
    has_vals = not bool(np.all(val_s == 1.0))

    # window edge counts -> Cmax
    win_starts = []   # per (core, w): slice into sorted arrays
    max_cnt = 0
    for c in range(NCORES):
        for w in range(NW):
            n0 = c * NPC + w * 128
            n1 = min(c * NPC + (w + 1) * 128, (c + 1) * NPC)
            a = np.searchsorted(tgt_s, n0)
            b = np.searchsorted(tgt_s, n1)
            win_starts.append((c, w, n0, a, b))
            max_cnt = max(max_cnt, b - a)
    # reserved last-slot-per-segment costs ~Cmax/8 slots per window
    Cmax = (max_cnt + 8 + 127) // 128
    while Cmax * 128 - ((Cmax + SEGC - 1) // SEGC + 1) < max_cnt:
        Cmax += 1
    TC = NW * Cmax                      # chunks per core
    TSEG = (TC + SEGC - 1) // SEGC      # gather segments per core

    src_i16 = np.zeros((NCORES, TC, 128), np.int16)
    tgt_i16 = np.zeros((NCORES, TC, 128), np.int16)
    tgtl = np.full((NCORES, NW, 128, Cmax), 200, np.uint8)
    vals = np.zeros((NCORES, NW, 128, Cmax), np.float32) if has_vals else None

    for (c, w, n0, a, b) in win_starts:
        cnt = b - a
        if cnt == 0:
            continue
        gc0 = w * Cmax
        # slot j = c_rel*128 + p, skipping reserved slots (global chunk
        # gc0+c_rel with (gc0+c_rel) % SEGC == SEGC-1 and p == 127)
        slots = np.arange(Cmax * 128)
        gcs = gc0 + slots // 128
        resv = ((gcs % SEGC) == SEGC - 1) & ((slots % 128) == 127)
        slots = slots[~resv][:cnt]
        assert len(slots) == cnt, (c, w, cnt, Cmax)
        crel = slots // 128
        p = slots % 128
        src_i16[c, gc0 + crel, p] = (src_s[a:b] - IDX_BIAS).astype(np.int16)
        tgt_i16[c, gc0 + crel, p] = (tgt_s[a:b] - IDX_BIAS).astype(np.int16)
        tgtl[c, w, p, crel] = (tgt_s[a:b] - n0).astype(np.uint8)
        if has_vals:
            vals[c, w, p, crel] = val_s[a:b]

    # compact idx arrays [16, TSEG*64]: seg s's idx j lives at
    # [j % 16, s*64 + j // 16]; replicated x8 across partitions on device
    def compact(arr):  # [TC, 128] -> [16, TSEG*64]
        # pad with 0 (NOT a negative value): the gather ucode trims trailing
        # negative indices, which would also drop real trailing edges whose
        # biased src index is negative in the final partial segment.
        flat = np.zeros(TSEG * SEG, np.int16)
        flat[:TC * 128] = arr.reshape(-1)
        return np.ascontiguousarray(
            flat.reshape(TSEG, 64, 16).transpose(2, 0, 1).reshape(16, TSEG * 64))

    idx_src = np.stack([compact(src_i16[c]) for c in range(NCORES)])
    idx_tgt = np.stack([compact(tgt_i16[c]) for c in range(NCORES)])

    # weights: wcat [128, 264] = [W (i->(h,o)) | ws | wt], f16
    W = np.asarray(weight, np.float32)              # [4, 128, 64]
    aw = np.asarray(att_weight, np.float32)         # [4, 128]
    ws = np.stack([W[h] @ aw[h, :OUT_CH] for h in range(N_HEADS)], 1)   # [128,4]
    wt = np.stack([W[h] @ aw[h, OUT_CH:] for h in range(N_HEADS)], 1)
    wcat = np.concatenate([W.transpose(1, 0, 2).reshape(IN_CH, HO), ws, wt],
                          1).astype(np.float16)

    # per-core x slice [128, NPC] f16 (channels-major)
    x_T = np.asarray(x_source, np.float32).T.astype(np.float16)  # [128, 50000]
    x_c = np.stack([np.ascontiguousarray(x_T[:, c * NPC:(c + 1) * NPC])
                    for c in range(NCORES)])

    tgtl = np.ascontiguousarray(tgtl.transpose(0, 2, 1, 3))  # [C,128,NW,Cmax]
    if has_vals:
        vals = np.ascontiguousarray(vals.transpose(0, 2, 1, 3))
    return dict(Cmax=Cmax, TC=TC, TSEG=TSEG, has_vals=has_vals, x_c=x_c,
                wcat=wcat, idx_src=idx_src, idx_tgt=idx_tgt, tgtl=tgtl,
                vals=vals)


def _build(Cmax, TC, TSEG, has_vals):
    import os
    KNW = int(os.environ.get("KNW", str(NW)))
    import concourse.bass as bass
    import concourse.tile as tile
    from concourse import bacc, mybir

    f32, f16, i16, i32, u8 = (mybir.dt.float32, mybir.dt.float16,
                              mybir.dt.int16, mybir.dt.int32, mybir.dt.uint8)
    Alu = mybir.AluOpType
    Act = mybir.ActivationFunctionType

    nc = bacc.Bacc("TRN2", target_bir_lowering=False, debug=False,
                   num_devices=NCORES, num_swdge_queues=1)
    x_c = nc.dram_tensor("x_c", [IN_CH, NPC], f16, kind="ExternalInput")
    wcat = nc.dram_tensor("wcat", [IN_CH, HO + 8], f16, kind="ExternalInput")
    idx_src = nc.dram_tensor("idx_src", [16, TSEG * 64], i16,
                             kind="ExternalInput")
    idx_tgt = nc.dram_tensor("idx_tgt", [16, TSEG * 64], i16,
                             kind="ExternalInput")
    tgtl_in = nc.dram_tensor("tgtl", [128, NW, Cmax], u8, kind="ExternalInput")
    if has_vals:
        vals_in = nc.dram_tensor("vals", [128, NW, Cmax], f32,
                                 kind="ExternalInput")
    out_d = nc.dram_tensor("out", [NPC, HO], f16, kind="ExternalOutput")

    with tile.TileContext(nc) as tc:
        with tc.tile_pool(name="dram", bufs=1, space="DRAM") as dpool:
            ag_in = dpool.tile([NPC, XROW], f16)
            xw = dpool.tile([N_NODES, XROW], f16)

            # ---------------- phase A (own nodes only) ----------------
            with tc.tile_pool(name="a_w", bufs=1) as cpool, \
                 tc.tile_pool(name="a_x", bufs=4) as xpool, \
                 tc.tile_pool(name="a_ps", bufs=4, space="PSUM") as apsum, \
                 tc.tile_pool(name="a_m", bufs=4) as mpool:
                wc = cpool.tile([128, HO + 8], f16)
                nc.sync.dma_start(wc[:], wcat[:])
                for i in range(NTA):
                    rows = min(128, NPC - i * 128)
                    xt = xpool.tile([128, 128], f16)
                    nc.sync.dma_start(xt[:, 0:rows],
                                      x_c[:, i * 128:i * 128 + rows])
                    ps = apsum.tile([128, HO + 8], f32)
                    nc.tensor.matmul(ps[0:rows, :], xt[:, 0:rows], wc[:])
                    m = mpool.tile([128, HO + 8], f16, tag="m")
                    nc.vector.tensor_copy(m[0:rows, :], ps[0:rows, 0:HO + 8])
                    nc.sync.dma_start(
                        ag_in[i * 128:i * 128 + rows, 0:HO + 8], m[0:rows, :])

            # on-device all-gather of the message table (node order)
            nc.gpsimd.collective_compute(
                "AllGather", mybir.AluOpType.bypass,
                replica_groups=[list(range(NCORES))],
                ins=[ag_in[:, :].opt()], outs=[xw[:, :].opt()])

            # ---------------- phase B ----------------
            with tc.tile_pool(name="b_c", bufs=1) as bconst, \
                 tc.tile_pool(name="b_g", bufs=8) as gpool, \
                 tc.tile_pool(name="b_t", bufs=8) as tpool, \
                 tc.tile_pool(name="b_z", bufs=3) as zpool, \
                 tc.tile_pool(name="b_oh", bufs=6) as ohpool, \
                 tc.tile_pool(name="b_ps", bufs=2, space="PSUM") as bpsum, \
                 tc.tile_pool(name="b_o", bufs=4) as opool:

                it32 = bconst.tile([128, 4 * 128], i32)
                nc.gpsimd.iota(it32[:], pattern=[[0, 4], [1, 128]],
                               channel_multiplier=0)
                iota4 = bconst.tile([128, 4, 128], f16)
                nc.vector.tensor_copy(iota4[:].rearrange("p a b -> p (a b)"),
                                      it32[:])
                bias_t = bconst.tile([128, 1], f32)
                nc.vector.memset(bias_t[:], EXP_BIAS)
                tlu = bconst.tile([128, NW, Cmax], u8)
                nc.sync.dma_start(tlu[:], tgtl_in[:])
                tl_all = bconst.tile([128, NW, Cmax], f16)
                nc.vector.tensor_copy(
                    tl_all[:].rearrange("p a b -> p (a b)"),
                    tlu[:].rearrange("p a b -> p (a b)"))
                if has_vals:
                    vv_all = bconst.tile([128, NW, Cmax], f32)
                    nc.sync.dma_start(vv_all[:], vals_in[:])
                # gather idx tiles, replicated to all 8 gpsimd core groups
                isrc = bconst.tile([128, TSEG * 64], i16)
                itgt = bconst.tile([128, TSEG * 64], i16)
                for k in range(8):
                    nc.scalar.dma_start(isrc[k * 16:(k + 1) * 16, :],
                                        idx_src[:, :])
                    nc.scalar.dma_start(itgt[k * 16:(k + 1) * 16, :],
                                        idx_tgt[:, :])

                tc.strict_bb_all_engine_barrier()

                seg_tiles = {}

                def get_seg(s):
                    if s not in seg_tiles:
                        g = gpool.tile([128, SEGC, XROW], f16)
                        nc.gpsimd.dma_gather(
                            g[:], xw[IDX_BIAS:, :],
                            isrc[:, s * 64:(s + 1) * 64], SEG, SEG,
                            XROW, queue_num=0)
                        tg = tpool.tile([128, SEGC, TROW], f16)
                        nc.gpsimd.dma_gather(
                            tg[:], xw[IDX_BIAS:, HO:HO + TROW],
                            itgt[:, s * 64:(s + 1) * 64], SEG, SEG,
                            TROW, elem_step=XROW, queue_num=0)
                        seg_tiles[s] = (g, tg)
                    return seg_tiles[s]

                def bc(apv, n):
                    return bass.AP(apv.tensor, apv.offset,
                                   list(apv.ap) + [[0, n]])

                for w in range(KNW):
                    rows = min(128, NPC - w * 128)
                    tl = tl_all[:, w, :]

                    gc0, gc1 = w * Cmax, (w + 1) * Cmax
                    segs = sorted({gc // SEGC for gc in range(gc0, gc1)})

                    # z = s + t (per segment range)
                    z = zpool.tile([128, Cmax, N_HEADS], f32, tag="z")
                    for s in segs:
                        lo = max(s * SEGC, gc0)
                        hi = min(s * SEGC + SEGC, gc1)
                        g, tg = get_seg(s)
                        nc.vector.tensor_tensor(
                            z[:, lo - gc0:hi - gc0, :],
                            g[:, lo - s * SEGC:hi - s * SEGC, HO:HO + 4],
                            tg[:, lo - s * SEGC:hi - s * SEGC, 4:8],
                            op=Alu.add)
                    # lrelu
                    zz = zpool.tile([128, Cmax, N_HEADS], f32, tag="zz")
                    nc.vector.scalar_tensor_tensor(
                        zz[:].rearrange("p c h -> p (c h)"),
                        z[:].rearrange("p c h -> p (c h)"), 0.01,
                        z[:].rearrange("p c h -> p (c h)"),
                        op0=Alu.mult, op1=Alu.max)
                    if has_vals:
                        nc.vector.tensor_tensor(
                            zz[:], zz[:], bc(vv_all[:, w, :], N_HEADS),
                            op=Alu.mult)
                    # p = exp(zz - 4)
                    p = zpool.tile([128, Cmax, N_HEADS], f16, tag="p")
                    nc.scalar.activation(p[:], zz[:], Act.Exp, bias=bias_t[:])

                    # rhs in-place: g.msg *= p ; g.s <- p
                    for s in segs:
                        lo = max(s * SEGC, gc0)
                        hi = min(s * SEGC + SEGC, gc1)
                        g, _ = get_seg(s)
                        gm = g[:, lo - s * SEGC:hi - s * SEGC, 0:HO].rearrange(
                            "p c (h o) -> p c h o", o=OUT_CH)
                        nc.vector.tensor_tensor(
                            gm, gm, bc(p[:, lo - gc0:hi - gc0, :], OUT_CH),
                            op=Alu.mult)
                        nc.vector.tensor_copy(
                            g[:, lo - s * SEGC:hi - s * SEGC, HO:HO + 4],
                            p[:, lo - gc0:hi - gc0, :])

                    ps = bpsum.tile([128, HO + 4], f32)
                    for cb in range(0, Cmax, 4):
                        nb = min(4, Cmax - cb)
                        oh = ohpool.tile([128, 4, 128], f16)
                        nc.vector.tensor_tensor(
                            oh[:, 0:nb, :], iota4[:, 0:nb, :],
                            bc(tl[:, cb:cb + nb], 128), op=Alu.is_equal)
                        for j in range(nb):
                            c = cb + j
                            gc = gc0 + c
                            g, _ = get_seg(gc // SEGC)
                            nc.tensor.matmul(
                                ps[:], oh[:, j, :],
                                g[:, gc % SEGC, 0:HO + 4],
                                start=(c == 0), stop=(c == Cmax - 1))

                    d = opool.tile([128, 4], f32, tag="d")
                    nc.vector.tensor_scalar_max(d[:], ps[:, HO:HO + 4], 1e-30)
                    r = opool.tile([128, 4], f32, tag="r")
                    nc.vector.reciprocal(r[:], d[:])
                    o = opool.tile([128, HO], f16, tag="o")
                    nc.vector.tensor_tensor(
                        o[:].rearrange("p (h q) -> p h q", q=OUT_CH),
                        ps[:, 0:HO].rearrange("p (h q) -> p h q", q=OUT_CH),
                        bc(r[:], OUT_CH), op=Alu.mult)
                    nc.sync.dma_start(out_d[w * 128:w * 128 + rows, :],
                                      o[0:rows, :])

    nc.finalize()
    return nc


_CACHE = {}


def kernel(x_source, edge_tgt, edge_src, edge_vals, weight, att_weight):
    from concourse import bass_utils

    prep = _host_prep(np.asarray(x_source), np.asarray(edge_tgt),
                      np.asarray(edge_src), np.asarray(edge_vals),
                      np.asarray(weight), np.asarray(att_weight))
    key = (prep["Cmax"], prep["TC"], prep["TSEG"], prep["has_vals"])
    if key not in _CACHE:
        _CACHE[key] = _build(*key)
    nc = _CACHE[key]

    in_maps = []
    for c in range(NCORES):
        m = {
            "x_c": prep["x_c"][c], "wcat": prep["wcat"],
            "idx_src": prep["idx_src"][c], "idx_tgt": prep["idx_tgt"][c],
            "tgtl": prep["tgtl"][c],
        }
        if prep["has_vals"]:
            m["vals"] = prep["vals"][c]
        in_maps.append(m)
    import time
    t0 = time.time()
    res = bass_utils.run_bass_kernel_spmd(nc, in_maps,
                                          core_ids=list(range(NCORES)))
    kernel.last_run_wall_s = time.time() - t0
    out = np.empty((N_NODES, HO), np.float32)
    for c in range(NCORES):
        out[c * NPC:(c + 1) * NPC, :] = res.results[c]["out"].astype(np.float32)
    return out


# revision 11
# speedup vs baseline: 1.1893x; 1.1893x over previous
"""Trainium2 Bass kernel for CAN multi-head message passing (GAT-style).

Strategy (vertex-cut by TARGET node, 8 cores):
  - Edges are sorted by target and sharded so core c owns target nodes
    [c*6250, (c+1)*6250). Each core fully computes its own output rows;
    no cross-core reduction is needed.
  - Phase A is SHARDED: core c computes x_msg = x @ W (4 heads) plus the
    per-node attention scalars s_n, t_n only for its own 6250 nodes
    (input = f16 x slice [128, 6250]), writes f16 rows
    [msg(256) | s(4) | t(4) | pad] (768B pitch) to a local DRAM chunk,
    then an on-device AllGather assembles the full 50000-row table on
    every core.  This keeps the host->device transfer per core at ~2.7MB
    instead of 34MB (the axon tunnel at ~40-50MB/s is the wall-clock
    bottleneck; device links are ~1000x faster).
  - Phase B: per 128-target-node window, per-edge rows are fetched with
    the GPSIMD dma_gather extended instruction.  The int16 index
    limitation is beaten by biasing indices by -32768 and pointing the
    gather base at row 32768: signed int16 offset arithmetic then
    addresses all 50k rows (verified on HW; the node table is stored in
    natural order).  Gathers are capped at 1024 indices per segment.
  - softmax (no max-subtraction needed: |z| <= ~10 for this data; a
    constant bias of -4 inside Exp guards fp16 range; constants cancel
    in softmax) and aggregation via one-hot matmuls: for each chunk of
    128 edges, a [128e x 128n] one-hot of local targets is built with a
    DVE is_equal and a single PE matmul accumulates both the weighted
    messages (256 cols) and the denominators (4 cols) into PSUM across
    all chunks of the window.
  - Gather index tiles are shipped compact ([16, nseg*64] int16) and
    replicated across the 8 gpsimd cores' partition groups on device;
    tgt-local ids ship as uint8; edge_vals are omitted when all-ones
    (the usual case); the output is written f16 and upcast on host.
"""
import sys
sys.path.insert(0, "/opt/trn_rl_repo")
import numpy as np

N_NODES = 50000
N_EDGES = 1600000
IN_CH = 128
OUT_CH = 64
N_HEADS = 4
HO = N_HEADS * OUT_CH          # 256
NCORES = 8
NPC = N_NODES // NCORES        # 6250 nodes per core
NW = 49                        # windows per core (48*128 + 106)
NTA = (NPC + 127) // 128       # phase-A tiles per core (49)
XROW = 384                     # fp16 elems per table row (768B): msg|s|t|pad
TROW = 128                     # fp16 elems per t-gather slice (256B)
SEG = 1024                     # max indices per dma_gather
SEGC = SEG // 128              # 8 chunks per segment
EXP_BIAS = -4.0
IDX_BIAS = 32768               # gather base at row 32768, idx = node - 32768


def _host_prep(x_source, edge_tgt, edge_src, edge_vals, weight, att_weight):
    perm = np.argsort(edge_tgt, kind="stable")
    tgt_s = np.asarray(edge_tgt)[perm].astype(np.int64)
    src_s = np.asarray(edge_src)[perm].astype(np.int64)
    val_s = np.asarray(edge_vals)[perm].astype(np.float32)
    has_vals = not bool(np.all(val_s == 1.0))

    # window edge counts -> Cmax
    win_starts = []   # per (core, w): slice into sorted arrays
    max_cnt = 0
    for c in range(NCORES):
        for w in range(NW):
            n0 = c * NPC + w * 128
            n1 = min(c * NPC + (w + 1) * 128, (c + 1) * NPC)
            a = np.searchsorted(tgt_s, n0)
            b = np.searchsorted(tgt_s, n1)
            win_starts.append((c, w, n0, a, b))
            max_cnt = max(max_cnt, b - a)
    # reserved last-slot-per-segment costs ~Cmax/8 slots per window
    Cmax = (max_cnt + 8 + 127) // 128
    while Cmax * 128 - ((Cmax + SEGC - 1) // SEGC + 1) < max_cnt:
        Cmax += 1
    TC = NW * Cmax                      # chunks per core
    TSEG = (TC + SEGC - 1) // SEGC      # gather segments per core

    src_i16 = np.zeros((NCORES, TC, 128), np.int16)
    tgt_i16 = np.zeros((NCORES, TC, 128), np.int16)
    tgtl = np.full((NCORES, NW, 128, Cmax), 200, np.uint8)
    vals = np.zeros((NCORES, NW, 128, Cmax), np.float32) if has_vals else None

    for (c, w, n0, a, b) in win_starts:
        cnt = b - a
        if cnt == 0:
            continue
        gc0 = w * Cmax
        # slot j = c_rel*128 + p, skipping reserved slots (global chunk
        # gc0+c_rel with (gc0+c_rel) % SEGC == SEGC-1 and p == 127)
        slots = np.arange(Cmax * 128)
        gcs = gc0 + slots // 128
        resv = ((gcs % SEGC) == SEGC - 1) & ((slots % 128) == 127)
        slots = slots[~resv][:cnt]
        assert len(slots) == cnt, (c, w, cnt, Cmax)
        crel = slots // 128
        p = slots % 128
        src_i16[c, gc0 + crel, p] = (src_s[a:b] - IDX_BIAS).astype(np.int16)
        tgt_i16[c, gc0 + crel, p] = (tgt_s[a:b] - IDX_BIAS).astype(np.int16)
        tgtl[c, w, p, crel] = (tgt_s[a:b] - n0).astype(np.uint8)
        if has_vals:
            vals[c, w, p, crel] = val_s[a:b]

    # compact idx arrays [16, TSEG*64]: seg s's idx j lives at
    # [j % 16, s*64 + j // 16]; replicated x8 across partitions on device
    def compact(arr):  # [TC, 128] -> [16, TSEG*64]
        # pad with 0 (NOT a negative value): the gather ucode trims trailing
        # negative indices, which would also drop real trailing edges whose
        # biased src index is negative in the final partial segment.
        flat = np.zeros(TSEG * SEG, np.int16)
        flat[:TC * 128] = arr.reshape(-1)
        return np.ascontiguousarray(
            flat.reshape(TSEG, 64, 16).transpose(2, 0, 1).reshape(16, TSEG * 64))

    # single [16, 2*TSEG*64] tensor: src idx block then tgt idx block
    idx_all = np.concatenate(
        [np.stack([compact(src_i16[c]) for c in range(NCORES)]),
         np.stack([compact(tgt_i16[c]) for c in range(NCORES)])], axis=2)

    # weights: wcat [128, 264] = [W (i->(h,o)) | ws | wt], f16
    W = np.asarray(weight, np.float32)              # [4, 128, 64]
    aw = np.asarray(att_weight, np.float32)         # [4, 128]
    ws = np.stack([W[h] @ aw[h, :OUT_CH] for h in range(N_HEADS)], 1)   # [128,4]
    wt = np.stack([W[h] @ aw[h, OUT_CH:] for h in range(N_HEADS)], 1)
    wcat = np.concatenate([W.transpose(1, 0, 2).reshape(IN_CH, HO), ws, wt],
                          1).astype(np.float16)

    # per-core x slice [128, NPC] f16 (channels-major) with wcat appended
    # as trailing columns -> one [128, NPC + 264] tensor
    x_T = np.asarray(x_source, np.float32).T.astype(np.float16)  # [128, 50000]
    x_c = np.stack([np.concatenate(
        [x_T[:, c * NPC:(c + 1) * NPC], wcat], axis=1)
        for c in range(NCORES)])

    tgtl = np.ascontiguousarray(tgtl.transpose(0, 2, 1, 3))  # [C,128,NW,Cmax]
    if has_vals:
        vals = np.ascontiguousarray(vals.transpose(0, 2, 1, 3))
    return dict(Cmax=Cmax, TC=TC, TSEG=TSEG, has_vals=has_vals, x_c=x_c,
                idx_all=idx_all, tgtl=tgtl, vals=vals)


def _build(Cmax, TC, TSEG, has_vals):
    import os
    KNW = int(os.environ.get("KNW", str(NW)))
    import concourse.bass as bass
    import concourse.tile as tile
    from concourse import bacc, mybir

    f32, f16, i16, i32, u8 = (mybir.dt.float32, mybir.dt.float16,
                              mybir.dt.int16, mybir.dt.int32, mybir.dt.uint8)
    Alu = mybir.AluOpType
    Act = mybir.ActivationFunctionType

    nc = bacc.Bacc("TRN2", target_bir_lowering=False, debug=False,
                   num_devices=NCORES, num_swdge_queues=1)
    x_c = nc.dram_tensor("x_c", [IN_CH, NPC + HO + 8], f16,
                         kind="ExternalInput")
    idx_all = nc.dram_tensor("idx_all", [16, 2 * TSEG * 64], i16,
                             kind="ExternalInput")
    tgtl_in = nc.dram_tensor("tgtl", [128, NW, Cmax], u8, kind="ExternalInput")
    if has_vals:
        vals_in = nc.dram_tensor("vals", [128, NW, Cmax], f32,
                                 kind="ExternalInput")
    out_d = nc.dram_tensor("out", [NPC, HO], f16, kind="ExternalOutput")

    with tile.TileContext(nc) as tc:
        with tc.tile_pool(name="dram", bufs=1, space="DRAM") as dpool:
            ag_in = dpool.tile([NPC, XROW], f16)
            xw = dpool.tile([N_NODES, XROW], f16)

            # ---------------- phase A (own nodes only) ----------------
            with tc.tile_pool(name="a_w", bufs=1) as cpool, \
                 tc.tile_pool(name="a_x", bufs=4) as xpool, \
                 tc.tile_pool(name="a_ps", bufs=4, space="PSUM") as apsum, \
                 tc.tile_pool(name="a_m", bufs=4) as mpool:
                wc = cpool.tile([128, HO + 8], f16)
                nc.sync.dma_start(wc[:], x_c[:, NPC:NPC + HO + 8])
                for i in range(NTA):
                    rows = min(128, NPC - i * 128)
                    xt = xpool.tile([128, 128], f16)
                    nc.sync.dma_start(xt[:, 0:rows],
                                      x_c[:, i * 128:i * 128 + rows])
                    ps = apsum.tile([128, HO + 8], f32)
                    nc.tensor.matmul(ps[0:rows, :], xt[:, 0:rows], wc[:])
                    m = mpool.tile([128, HO + 8], f16, tag="m")
                    nc.vector.tensor_copy(m[0:rows, :], ps[0:rows, 0:HO + 8])
                    nc.sync.dma_start(
                        ag_in[i * 128:i * 128 + rows, 0:HO + 8], m[0:rows, :])

            # on-device all-gather of the message table (node order)
            nc.gpsimd.collective_compute(
                "AllGather", mybir.AluOpType.bypass,
                replica_groups=[list(range(NCORES))],
                ins=[ag_in[:, :].opt()], outs=[xw[:, :].opt()])

            # ---------------- phase B ----------------
            with tc.tile_pool(name="b_c", bufs=1) as bconst, \
                 tc.tile_pool(name="b_g", bufs=8) as gpool, \
                 tc.tile_pool(name="b_t", bufs=8) as tpool, \
                 tc.tile_pool(name="b_z", bufs=3) as zpool, \
                 tc.tile_pool(name="b_oh", bufs=6) as ohpool, \
                 tc.tile_pool(name="b_ps", bufs=2, space="PSUM") as bpsum, \
                 tc.tile_pool(name="b_o", bufs=4) as opool:

                it32 = bconst.tile([128, 4 * 128], i32)
                nc.gpsimd.iota(it32[:], pattern=[[0, 4], [1, 128]],
                               channel_multiplier=0)
                iota4 = bconst.tile([128, 4, 128], f16)
                nc.vector.tensor_copy(iota4[:].rearrange("p a b -> p (a b)"),
                                      it32[:])
                bias_t = bconst.tile([128, 1], f32)
                nc.vector.memset(bias_t[:], EXP_BIAS)
                tlu = bconst.tile([128, NW, Cmax], u8)
                nc.sync.dma_start(tlu[:], tgtl_in[:])
                tl_all = bconst.tile([128, NW, Cmax], f16)
                nc.vector.tensor_copy(
                    tl_all[:].rearrange("p a b -> p (a b)"),
                    tlu[:].rearrange("p a b -> p (a b)"))
                if has_vals:
                    vv_all = bconst.tile([128, NW, Cmax], f32)
                    nc.sync.dma_start(vv_all[:], vals_in[:])
                # gather idx tiles, replicated to all 8 gpsimd core groups
                iall = bconst.tile([128, 2 * TSEG * 64], i16)
                for k in range(8):
                    nc.scalar.dma_start(iall[k * 16:(k + 1) * 16, :],
                                        idx_all[:, :])

                tc.strict_bb_all_engine_barrier()

                seg_tiles = {}

                def get_seg(s):
                    if s not in seg_tiles:
                        g = gpool.tile([128, SEGC, XROW], f16)
                        nc.gpsimd.dma_gather(
                            g[:], xw[IDX_BIAS:, :],
                            iall[:, s * 64:(s + 1) * 64], SEG, SEG,
                            XROW, queue_num=0)
                        tg = tpool.tile([128, SEGC, TROW], f16)
                        nc.gpsimd.dma_gather(
                            tg[:], xw[IDX_BIAS:, HO:HO + TROW],
                            iall[:, TSEG * 64 + s * 64:
                                 TSEG * 64 + (s + 1) * 64], SEG, SEG,
                            TROW, elem_step=XROW, queue_num=0)
                        seg_tiles[s] = (g, tg)
                    return seg_tiles[s]

                def bc(apv, n):
                    return bass.AP(apv.tensor, apv.offset,
                                   list(apv.ap) + [[0, n]])

                for w in range(KNW):
                    rows = min(128, NPC - w * 128)
                    tl = tl_all[:, w, :]

                    gc0, gc1 = w * Cmax, (w + 1) * Cmax
                    segs = sorted({gc // SEGC for gc in range(gc0, gc1)})

                    # z = s + t (per segment range)
                    z = zpool.tile([128, Cmax, N_HEADS], f32, tag="z")
                    for s in segs:
                        lo = max(s * SEGC, gc0)
                        hi = min(s * SEGC + SEGC, gc1)
                        g, tg = get_seg(s)
                        nc.vector.tensor_tensor(
                            z[:, lo - gc0:hi - gc0, :],
                            g[:, lo - s * SEGC:hi - s * SEGC, HO:HO + 4],
                            tg[:, lo - s * SEGC:hi - s * SEGC, 4:8],
                            op=Alu.add)
                    # lrelu
                    zz = zpool.tile([128, Cmax, N_HEADS], f32, tag="zz")
                    nc.vector.scalar_tensor_tensor(
                        zz[:].rearrange("p c h -> p (c h)"),
                        z[:].rearrange("p c h -> p (c h)"), 0.01,
                        z[:].rearrange("p c h -> p (c h)"),
                        op0=Alu.mult, op1=Alu.max)
                    if has_vals:
                        nc.vector.tensor_tensor(
                            zz[:], zz[:], bc(vv_all[:, w, :], N_HEADS),
                            op=Alu.mult)
                    # p = exp(zz - 4)
                    p = zpool.tile([128, Cmax, N_HEADS], f16, tag="p")
                    nc.scalar.activation(p[:], zz[:], Act.Exp, bias=bias_t[:])

                    # rhs in-place: g.msg *= p ; g.s <- p
                    for s in segs:
                        lo = max(s * SEGC, gc0)
                        hi = min(s * SEGC + SEGC, gc1)
                        g, _ = get_seg(s)
                        gm = g[:, lo - s * SEGC:hi - s * SEGC, 0:HO].rearrange(
                            "p c (h o) -> p c h o", o=OUT_CH)
                        nc.vector.tensor_tensor(
                            gm, gm, bc(p[:, lo - gc0:hi - gc0, :], OUT_CH),
                            op=Alu.mult)
                        nc.vector.tensor_copy(
                            g[:, lo - s * SEGC:hi - s * SEGC, HO:HO + 4],
                            p[:, lo - gc0:hi - gc0, :])

                    ps = bpsum.tile([128, HO + 4], f32)
                    for cb in range(0, Cmax, 4):
                        nb = min(4, Cmax - cb)
                        oh = ohpool.tile([128, 4, 128], f16)
                        nc.vector.tensor_tensor(
                            oh[:, 0:nb, :], iota4[:, 0:nb, :],
                            bc(tl[:, cb:cb + nb], 128), op=Alu.is_equal)
                        for j in range(nb):
                            c = cb + j
                            gc = gc0 + c
                            g, _ = get_seg(gc // SEGC)
                            nc.tensor.matmul(
                                ps[:], oh[:, j, :],
                                g[:, gc % SEGC, 0:HO + 4],
                                start=(c == 0), stop=(c == Cmax - 1))

                    d = opool.tile([128, 4], f32, tag="d")
                    nc.vector.tensor_scalar_max(d[:], ps[:, HO:HO + 4], 1e-30)
                    r = opool.tile([128, 4], f32, tag="r")
                    nc.vector.reciprocal(r[:], d[:])
                    o = opool.tile([128, HO], f16, tag="o")
                    nc.vector.tensor_tensor(
                        o[:].rearrange("p (h q) -> p h q", q=OUT_CH),
                        ps[:, 0:HO].rearrange("p (h q) -> p h q", q=OUT_CH),
                        bc(r[:], OUT_CH), op=Alu.mult)
                    nc.sync.dma_start(out_d[w * 128:w * 128 + rows, :],
                                      o[0:rows, :])

    nc.finalize()
    return nc


_CACHE = {}


def kernel(x_source, edge_tgt, edge_src, edge_vals, weight, att_weight):
    from concourse import bass_utils

    prep = _host_prep(np.asarray(x_source), np.asarray(edge_tgt),
                      np.asarray(edge_src), np.asarray(edge_vals),
                      np.asarray(weight), np.asarray(att_weight))
    key = (prep["Cmax"], prep["TC"], prep["TSEG"], prep["has_vals"])
    if key not in _CACHE:
        _CACHE[key] = _build(*key)
    nc = _CACHE[key]

    in_maps = []
    for c in range(NCORES):
        m = {
            "x_c": prep["x_c"][c], "idx_all": prep["idx_all"][c],
            "tgtl": prep["tgtl"][c],
        }
        if prep["has_vals"]:
            m["vals"] = prep["vals"][c]
        in_maps.append(m)
    import time
    t0 = time.time()
    res = bass_utils.run_bass_kernel_spmd(nc, in_maps,
                                          core_ids=list(range(NCORES)))
    kernel.last_run_wall_s = time.time() - t0
    out = np.empty((N_NODES, HO), np.float32)
    for c in range(NCORES):
        out[c * NPC:(c + 1) * NPC, :] = res.results[c]["out"].astype(np.float32)
    return out


# revision 12
# speedup vs baseline: 1.3040x; 1.0965x over previous
"""Trainium2 Bass kernel for CAN multi-head message passing (GAT-style).

Strategy (vertex-cut by TARGET node, 8 cores):
  - Edges are sorted by target and sharded so core c owns target nodes
    [c*6250, (c+1)*6250). Each core fully computes its own output rows;
    no cross-core reduction is needed.
  - Phase A is SHARDED: core c computes x_msg = x @ W (4 heads) plus the
    per-node attention scalars s_n, t_n only for its own 6250 nodes
    (input = f16 x slice [128, 6250]), writes f16 rows
    [msg(256) | s(4) | t(4) | pad] (768B pitch) to a local DRAM chunk,
    then an on-device AllGather assembles the full 50000-row table on
    every core.  This keeps the host->device transfer per core at ~2.7MB
    instead of 34MB (the axon tunnel at ~40-50MB/s is the wall-clock
    bottleneck; device links are ~1000x faster).
  - Phase B: per 128-target-node window, per-edge rows are fetched with
    the GPSIMD dma_gather extended instruction.  The int16 index
    limitation is beaten by biasing indices by -32768 and pointing the
    gather base at row 32768: signed int16 offset arithmetic then
    addresses all 50k rows (verified on HW; the node table is stored in
    natural order).  Gathers are capped at 1024 indices per segment.
  - softmax (no max-subtraction needed: |z| <= ~10 for this data; a
    constant bias of -4 inside Exp guards fp16 range; constants cancel
    in softmax) and aggregation via one-hot matmuls: for each chunk of
    128 edges, a [128e x 128n] one-hot of local targets is built with a
    DVE is_equal and a single PE matmul accumulates both the weighted
    messages (256 cols) and the denominators (4 cols) into PSUM across
    all chunks of the window.
  - Host->device payload is minimized and packed into 3 tensors per core
    (the axon tunnel has ~70ms/put latency + ~50-70MB/s bandwidth):
    x slice with wcat appended (f16 [128, 6514], 1.7MB), src+tgt gather
    indices compact ([16, 2*nseg*64] int16, 0.86MB; replicated across
    the 8 gpsimd cores' partition groups on device with 8 small DMAs),
    and tgt-local ids as uint8 (0.21MB).  edge_vals are omitted when
    all-ones (the spec fill); the output is written f16 and upcast on
    host.  Total moved per call: ~19MB up + 25.6MB zero-donation up
    (output buffers) + 25.6MB down.
  - Gather segments are padded with index 0, NOT a negative value: the
    gather ucode trims trailing negative indices (and a fully-negative
    trailing segment deadlocks the descriptor rings).
"""
import sys
sys.path.insert(0, "/opt/trn_rl_repo")
import numpy as np

N_NODES = 50000
N_EDGES = 1600000
IN_CH = 128
OUT_CH = 64
N_HEADS = 4
HO = N_HEADS * OUT_CH          # 256
NCORES = 8
NPC = N_NODES // NCORES        # 6250 nodes per core
NW = 49                        # windows per core (48*128 + 106)
NTA = (NPC + 127) // 128       # phase-A tiles per core (49)
XROW = 384                     # fp16 elems per table row (768B): msg|s|t|pad
TROW = 128                     # fp16 elems per t-gather slice (256B)
SEG = 1024                     # max indices per dma_gather
SEGC = SEG // 128              # 8 chunks per segment
EXP_BIAS = -4.0
IDX_BIAS = 32768               # gather base at row 32768, idx = node - 32768


def _host_prep(x_source, edge_tgt, edge_src, edge_vals, weight, att_weight):
    perm = np.argsort(edge_tgt, kind="stable")
    tgt_s = np.asarray(edge_tgt)[perm].astype(np.int64)
    src_s = np.asarray(edge_src)[perm].astype(np.int64)
    val_s = np.asarray(edge_vals)[perm].astype(np.float32)
    has_vals = not bool(np.all(val_s == 1.0))

    # window edge counts -> Cmax
    win_starts = []   # per (core, w): slice into sorted arrays
    max_cnt = 0
    for c in range(NCORES):
        for w in range(NW):
            n0 = c * NPC + w * 128
            n1 = min(c * NPC + (w + 1) * 128, (c + 1) * NPC)
            a = np.searchsorted(tgt_s, n0)
            b = np.searchsorted(tgt_s, n1)
            win_starts.append((c, w, n0, a, b))
            max_cnt = max(max_cnt, b - a)
    # reserved last-slot-per-segment costs ~Cmax/8 slots per window
    Cmax = (max_cnt + 8 + 127) // 128
    while Cmax * 128 - ((Cmax + SEGC - 1) // SEGC + 1) < max_cnt:
        Cmax += 1
    TC = NW * Cmax                      # chunks per core
    TSEG = (TC + SEGC - 1) // SEGC      # gather segments per core

    src_i16 = np.zeros((NCORES, TC, 128), np.int16)
    tgt_i16 = np.zeros((NCORES, TC, 128), np.int16)
    tgtl = np.full((NCORES, NW, 128, Cmax), 200, np.uint8)
    vals = np.zeros((NCORES, NW, 128, Cmax), np.float32) if has_vals else None

    for (c, w, n0, a, b) in win_starts:
        cnt = b - a
        if cnt == 0:
            continue
        gc0 = w * Cmax
        # slot j = c_rel*128 + p, skipping reserved slots (global chunk
        # gc0+c_rel with (gc0+c_rel) % SEGC == SEGC-1 and p == 127)
        slots = np.arange(Cmax * 128)
        gcs = gc0 + slots // 128
        resv = ((gcs % SEGC) == SEGC - 1) & ((slots % 128) == 127)
        slots = slots[~resv][:cnt]
        assert len(slots) == cnt, (c, w, cnt, Cmax)
        crel = slots // 128
        p = slots % 128
        src_i16[c, gc0 + crel, p] = (src_s[a:b] - IDX_BIAS).astype(np.int16)
        tgt_i16[c, gc0 + crel, p] = (tgt_s[a:b] - IDX_BIAS).astype(np.int16)
        tgtl[c, w, p, crel] = (tgt_s[a:b] - n0).astype(np.uint8)
        if has_vals:
            vals[c, w, p, crel] = val_s[a:b]

    # compact idx arrays [16, TSEG*64]: seg s's idx j lives at
    # [j % 16, s*64 + j // 16]; replicated x8 across partitions on device
    def compact(arr):  # [TC, 128] -> [16, TSEG*64]
        # pad with 0 (NOT a negative value): the gather ucode trims trailing
        # negative indices, which would also drop real trailing edges whose
        # biased src index is negative in the final partial segment.
        flat = np.zeros(TSEG * SEG, np.int16)
        flat[:TC * 128] = arr.reshape(-1)
        return np.ascontiguousarray(
            flat.reshape(TSEG, 64, 16).transpose(2, 0, 1).reshape(16, TSEG * 64))

    # single [16, 2*TSEG*64] tensor: src idx block then tgt idx block
    idx_all = np.concatenate(
        [np.stack([compact(src_i16[c]) for c in range(NCORES)]),
         np.stack([compact(tgt_i16[c]) for c in range(NCORES)])], axis=2)

    # weights: wcat [128, 264] = [W (i->(h,o)) | ws | wt], f16
    W = np.asarray(weight, np.float32)              # [4, 128, 64]
    aw = np.asarray(att_weight, np.float32)         # [4, 128]
    ws = np.stack([W[h] @ aw[h, :OUT_CH] for h in range(N_HEADS)], 1)   # [128,4]
    wt = np.stack([W[h] @ aw[h, OUT_CH:] for h in range(N_HEADS)], 1)
    wcat = np.concatenate([W.transpose(1, 0, 2).reshape(IN_CH, HO), ws, wt],
                          1).astype(np.float16)

    # per-core x slice [128, NPC] f16 (channels-major) with wcat appended
    # as trailing columns -> one [128, NPC + 264] tensor
    x_T = np.asarray(x_source, np.float32).T.astype(np.float16)  # [128, 50000]
    x_c = np.stack([np.concatenate(
        [x_T[:, c * NPC:(c + 1) * NPC], wcat], axis=1)
        for c in range(NCORES)])

    tgtl = np.ascontiguousarray(tgtl.transpose(0, 2, 1, 3))  # [C,128,NW,Cmax]
    if has_vals:
        vals = np.ascontiguousarray(vals.transpose(0, 2, 1, 3))
    return dict(Cmax=Cmax, TC=TC, TSEG=TSEG, has_vals=has_vals, x_c=x_c,
                idx_all=idx_all, tgtl=tgtl, vals=vals)


def _build(Cmax, TC, TSEG, has_vals):
    import os
    KNW = int(os.environ.get("KNW", str(NW)))
    import concourse.bass as bass
    import concourse.tile as tile
    from concourse import bacc, mybir

    f32, f16, i16, i32, u8 = (mybir.dt.float32, mybir.dt.float16,
                              mybir.dt.int16, mybir.dt.int32, mybir.dt.uint8)
    Alu = mybir.AluOpType
    Act = mybir.ActivationFunctionType

    nc = bacc.Bacc("TRN2", target_bir_lowering=False, debug=False,
                   num_devices=NCORES, num_swdge_queues=1)
    x_c = nc.dram_tensor("x_c", [IN_CH, NPC + HO + 8], f16,
                         kind="ExternalInput")
    idx_all = nc.dram_tensor("idx_all", [16, 2 * TSEG * 64], i16,
                             kind="ExternalInput")
    tgtl_in = nc.dram_tensor("tgtl", [128, NW, Cmax], u8, kind="ExternalInput")
    if has_vals:
        vals_in = nc.dram_tensor("vals", [128, NW, Cmax], f32,
                                 kind="ExternalInput")
    out_d = nc.dram_tensor("out", [NPC, HO], f16, kind="ExternalOutput")

    with tile.TileContext(nc) as tc:
        with tc.tile_pool(name="dram", bufs=1, space="DRAM") as dpool:
            ag_in = dpool.tile([NPC, XROW], f16)
            xw = dpool.tile([N_NODES, XROW], f16)

            # ---------------- phase A (own nodes only) ----------------
            with tc.tile_pool(name="a_w", bufs=1) as cpool, \
                 tc.tile_pool(name="a_x", bufs=4) as xpool, \
                 tc.tile_pool(name="a_ps", bufs=4, space="PSUM") as apsum, \
                 tc.tile_pool(name="a_m", bufs=4) as mpool:
                wc = cpool.tile([128, HO + 8], f16)
                nc.sync.dma_start(wc[:], x_c[:, NPC:NPC + HO + 8])
                for i in range(NTA):
                    rows = min(128, NPC - i * 128)
                    xt = xpool.tile([128, 128], f16)
                    nc.sync.dma_start(xt[:, 0:rows],
                                      x_c[:, i * 128:i * 128 + rows])
                    ps = apsum.tile([128, HO + 8], f32)
                    nc.tensor.matmul(ps[0:rows, :], xt[:, 0:rows], wc[:])
                    m = mpool.tile([128, HO + 8], f16, tag="m")
                    nc.vector.tensor_copy(m[0:rows, :], ps[0:rows, 0:HO + 8])
                    nc.sync.dma_start(
                        ag_in[i * 128:i * 128 + rows, 0:HO + 8], m[0:rows, :])

            # on-device all-gather of the message table (node order)
            nc.gpsimd.collective_compute(
                "AllGather", mybir.AluOpType.bypass,
                replica_groups=[list(range(NCORES))],
                ins=[ag_in[:, :].opt()], outs=[xw[:, :].opt()])

            # ---------------- phase B ----------------
            with tc.tile_pool(name="b_c", bufs=1) as bconst, \
                 tc.tile_pool(name="b_g", bufs=8) as gpool, \
                 tc.tile_pool(name="b_t", bufs=8) as tpool, \
                 tc.tile_pool(name="b_z", bufs=3) as zpool, \
                 tc.tile_pool(name="b_oh", bufs=6) as ohpool, \
                 tc.tile_pool(name="b_ps", bufs=2, space="PSUM") as bpsum, \
                 tc.tile_pool(name="b_o", bufs=4) as opool:

                it32 = bconst.tile([128, 4 * 128], i32)
                nc.gpsimd.iota(it32[:], pattern=[[0, 4], [1, 128]],
                               channel_multiplier=0)
                iota4 = bconst.tile([128, 4, 128], f16)
                nc.vector.tensor_copy(iota4[:].rearrange("p a b -> p (a b)"),
                                      it32[:])
                bias_t = bconst.tile([128, 1], f32)
                nc.vector.memset(bias_t[:], EXP_BIAS)
                tlu = bconst.tile([128, NW, Cmax], u8)
                nc.sync.dma_start(tlu[:], tgtl_in[:])
                tl_all = bconst.tile([128, NW, Cmax], f16)
                nc.vector.tensor_copy(
                    tl_all[:].rearrange("p a b -> p (a b)"),
                    tlu[:].rearrange("p a b -> p (a b)"))
                if has_vals:
                    vv_all = bconst.tile([128, NW, Cmax], f32)
                    nc.sync.dma_start(vv_all[:], vals_in[:])
                # gather idx tiles, replicated to all 8 gpsimd core groups
                iall = bconst.tile([128, 2 * TSEG * 64], i16)
                for k in range(8):
                    nc.scalar.dma_start(iall[k * 16:(k + 1) * 16, :],
                                        idx_all[:, :])

                tc.strict_bb_all_engine_barrier()

                seg_tiles = {}

                def get_seg(s):
                    if s not in seg_tiles:
                        g = gpool.tile([128, SEGC, XROW], f16)
                        nc.gpsimd.dma_gather(
                            g[:], xw[IDX_BIAS:, :],
                            iall[:, s * 64:(s + 1) * 64], SEG, SEG,
                            XROW, queue_num=0)
                        tg = tpool.tile([128, SEGC, TROW], f16)
                        nc.gpsimd.dma_gather(
                            tg[:], xw[IDX_BIAS:, HO:HO + TROW],
                            iall[:, TSEG * 64 + s * 64:
                                 TSEG * 64 + (s + 1) * 64], SEG, SEG,
                            TROW, elem_step=XROW, queue_num=0)
                        seg_tiles[s] = (g, tg)
                    return seg_tiles[s]

                def bc(apv, n):
                    return bass.AP(apv.tensor, apv.offset,
                                   list(apv.ap) + [[0, n]])

                for w in range(KNW):
                    rows = min(128, NPC - w * 128)
                    tl = tl_all[:, w, :]

                    gc0, gc1 = w * Cmax, (w + 1) * Cmax
                    segs = sorted({gc // SEGC for gc in range(gc0, gc1)})

                    # z = s + t (per segment range)
                    z = zpool.tile([128, Cmax, N_HEADS], f32, tag="z")
                    for s in segs:
                        lo = max(s * SEGC, gc0)
                        hi = min(s * SEGC + SEGC, gc1)
                        g, tg = get_seg(s)
                        nc.vector.tensor_tensor(
                            z[:, lo - gc0:hi - gc0, :],
                            g[:, lo - s * SEGC:hi - s * SEGC, HO:HO + 4],
                            tg[:, lo - s * SEGC:hi - s * SEGC, 4:8],
                            op=Alu.add)
                    # lrelu
                    zz = zpool.tile([128, Cmax, N_HEADS], f32, tag="zz")
                    nc.vector.scalar_tensor_tensor(
                        zz[:].rearrange("p c h -> p (c h)"),
                        z[:].rearrange("p c h -> p (c h)"), 0.01,
                        z[:].rearrange("p c h -> p (c h)"),
                        op0=Alu.mult, op1=Alu.max)
                    if has_vals:
                        nc.vector.tensor_tensor(
                            zz[:], zz[:], bc(vv_all[:, w, :], N_HEADS),
                            op=Alu.mult)
                    # p = exp(zz - 4)
                    p = zpool.tile([128, Cmax, N_HEADS], f16, tag="p")
                    nc.scalar.activation(p[:], zz[:], Act.Exp, bias=bias_t[:])

                    # rhs in-place: g.msg *= p ; g.s <- p
                    for s in segs:
                        lo = max(s * SEGC, gc0)
                        hi = min(s * SEGC + SEGC, gc1)
                        g, _ = get_seg(s)
                        gm = g[:, lo - s * SEGC:hi - s * SEGC, 0:HO].rearrange(
                            "p c (h o) -> p c h o", o=OUT_CH)
                        nc.vector.tensor_tensor(
                            gm, gm, bc(p[:, lo - gc0:hi - gc0, :], OUT_CH),
                            op=Alu.mult)
                        nc.vector.tensor_copy(
                            g[:, lo - s * SEGC:hi - s * SEGC, HO:HO + 4],
                            p[:, lo - gc0:hi - gc0, :])

                    ps = bpsum.tile([128, HO + 4], f32)
                    for cb in range(0, Cmax, 4):
                        nb = min(4, Cmax - cb)
                        oh = ohpool.tile([128, 4, 128], f16)
                        nc.vector.tensor_tensor(
                            oh[:, 0:nb, :], iota4[:, 0:nb, :],
                            bc(tl[:, cb:cb + nb], 128), op=Alu.is_equal)
                        for j in range(nb):
                            c = cb + j
                            gc = gc0 + c
                            g, _ = get_seg(gc // SEGC)
                            nc.tensor.matmul(
                                ps[:], oh[:, j, :],
                                g[:, gc % SEGC, 0:HO + 4],
                                start=(c == 0), stop=(c == Cmax - 1))

                    d = opool.tile([128, 4], f32, tag="d")
                    nc.vector.tensor_scalar_max(d[:], ps[:, HO:HO + 4], 1e-30)
                    r = opool.tile([128, 4], f32, tag="r")
                    nc.vector.reciprocal(r[:], d[:])
                    o = opool.tile([128, HO], f16, tag="o")
                    nc.vector.tensor_tensor(
                        o[:].rearrange("p (h q) -> p h q", q=OUT_CH),
                        ps[:, 0:HO].rearrange("p (h q) -> p h q", q=OUT_CH),
                        bc(r[:], OUT_CH), op=Alu.mult)
                    nc.sync.dma_start(out_d[w * 128:w * 128 + rows, :],
                                      o[0:rows, :])

    nc.finalize()
    return nc


_CACHE = {}


def kernel(x_source, edge_tgt, edge_src, edge_vals, weight, att_weight):
    from concourse import bass_utils

    prep = _host_prep(np.asarray(x_source), np.asarray(edge_tgt),
                      np.asarray(edge_src), np.asarray(edge_vals),
                      np.asarray(weight), np.asarray(att_weight))
    key = (prep["Cmax"], prep["TC"], prep["TSEG"], prep["has_vals"])
    if key not in _CACHE:
        _CACHE[key] = _build(*key)
    nc = _CACHE[key]

    in_maps = []
    for c in range(NCORES):
        m = {
            "x_c": prep["x_c"][c], "idx_all": prep["idx_all"][c],
            "tgtl": prep["tgtl"][c],
        }
        if prep["has_vals"]:
            m["vals"] = prep["vals"][c]
        in_maps.append(m)
    import time
    t0 = time.time()
    res = bass_utils.run_bass_kernel_spmd(nc, in_maps,
                                          core_ids=list(range(NCORES)))
    kernel.last_run_wall_s = time.time() - t0
    out = np.empty((N_NODES, HO), np.float32)
    for c in range(NCORES):
        out[c * NPC:(c + 1) * NPC, :] = res.results[c]["out"].astype(np.float32)
    return out
